# revision 15
# baseline (speedup 1.0000x reference)
"""Trainium2 Bass kernel for nn_CppnPotentialCA (CPPN potential cellular automaton).

Reference computation (shapes hardcoded):
  x       [1,96,96,96,9] f32   potential field
  kernels [64,5,5,5]     f32   cross-channel conv kernels (normalized by sum)
  m, s    [64]           f32   Gaussian growth center / width
  T       []             f32   temperature
  c0, c1  [64]           i32   source / target channel per kernel pair

  kn   = kernels / sum(kernels)                  (per kernel, if sum > 0)
  pot  = conv3d_valid(wrap_pad(x)[c0[p]], kn[p]) for each pair p   [64,96,96,96]
  g    = exp(-(pot-m)^2 / (2 s^2)) * 2 - 1
  out  = clip(x + segment_sum(g, c1)/T, 0, 10)

Sharding: data-parallel over the depth (SX) axis, 12 output z-planes per core,
halo of 2 handled by host-side toroidal padding (no device collectives).

Device mapping (per core):
  - conv as dense matmul: contraction = (channel 9, z-plane-pair 2, dy 5) = 90
    SBUF partitions; dx in {0..4} via free-dim AP offsets; 3 z-chunks x 5 dx =
    15 accumulating matmuls per PSUM tile; output partitions = (pair 64, zo 2)
    = 128; free dim = 4 y-rows x 96 = 384.
  - growth via two ScalarE activation passes:
      u  = Square(pot * 1/(s*sqrt2) - m/(s*sqrt2)) = (pot-m)^2/(2 s^2)
      g' = Exp(-u + ln(2/T)) = (2/T) exp(-(pot-m)^2/(2 s^2))     (fp16)
  - segment-sum over pairs as matmul with 0/1 matrix E[(p,zo),(c,zo)] -> PSUM
  - out = clip(xmod + seg, 0, 10) on VectorE, where xmod = x - cnt_c/T is
    precomputed on host (folds the "*2-1" count term).
All inputs are runtime data (weights/E/scales are DRAM inputs), so the
compiled program is value-independent.
"""

import numpy as np

C = 9        # channels
S = 96       # spatial side
P = 64       # kernel pairs
K = 5        # kernel side
PAD = 2
MAXP = 10.0
NCORES = 8
ZS = S // NCORES          # output z-planes per core = 12
SLAB = ZS + 2 * PAD       # input z-planes per core = 16
XW = S + 2 * PAD          # padded y/x width = 100
YB = 4                    # y-rows per PSUM tile
NYB = S // YB             # 24 y-blocks
FREE = YB * S             # matmul free dim = 384
NGRP = 4                  # y-block groups per z-pair
GRP = NYB // NGRP         # 6 y-blocks per group
NZP = ZS // 2             # 6 z-pairs per core
KROW = C * 2 * K          # contraction rows = 90
NSET = 3 * K              # 15 weight sets (z-chunk j x dx)


def _build_nc():
    from contextlib import ExitStack

    import concourse.bass as bass
    import concourse.tile as tile
    from concourse import bacc, mybir

    f32 = mybir.dt.float32
    f16 = mybir.dt.float16
    AF = mybir.ActivationFunctionType
    ALU = mybir.AluOpType

    nc = bacc.Bacc("TRN2", target_bir_lowering=False, debug=False,
                   num_devices=NCORES)

    # Pre-expanded im2col input: xim[q, (ci,t,dy), 9600] where row (ci,t,dy) =
    # wrap-padded plane (2q+t) of channel ci, rows dy..dy+95 flattened.
    xim_d = nc.dram_tensor("xim", [SLAB // 2, KROW, S * XW], f16,
                           kind="ExternalInput")
    xmod_d = nc.dram_tensor("xmod", [C, ZS, S, S], f32, kind="ExternalInput")
    w_d = nc.dram_tensor("wmat", [KROW, NSET, 128], f16, kind="ExternalInput")
    e_d = nc.dram_tensor("emat", [128, 2 * C], f16, kind="ExternalInput")
    sv_d = nc.dram_tensor("svec", [128, 1], f32, kind="ExternalInput")
    bv_d = nc.dram_tensor("bvec", [128, 1], f32, kind="ExternalInput")
    b2_d = nc.dram_tensor("b2vec", [128, 1], f32, kind="ExternalInput")
    out_d = nc.dram_tensor("out", [C, ZS, S, S], f32, kind="ExternalOutput")

    XM_C = ZS * S * S        # xmod channel stride
    XM_Z = S * S             # xmod z stride

    with tile.TileContext(nc) as tc, ExitStack() as ctx:
        consts = ctx.enter_context(tc.tile_pool(name="consts", bufs=1))
        rpool = ctx.enter_context(tc.tile_pool(name="rtiles", bufs=4))
        u2pool = ctx.enter_context(tc.tile_pool(name="u2", bufs=3))
        gpool = ctx.enter_context(tc.tile_pool(name="growth", bufs=3))
        xpool = ctx.enter_context(tc.tile_pool(name="xin", bufs=2))
        opool = ctx.enter_context(tc.tile_pool(name="oout", bufs=2))
        pconv = ctx.enter_context(tc.tile_pool(name="pconv", bufs=6, space="PSUM"))
        pseg = ctx.enter_context(tc.tile_pool(name="pseg", bufs=2, space="PSUM"))

        # Resident constants
        w_sb = consts.tile([KROW, NSET * 128], f16)
        nc.sync.dma_start(w_sb[:, :], w_d.ap().rearrange("k s m -> k (s m)"))
        e_sb = consts.tile([128, 2 * C], f16)
        nc.sync.dma_start(e_sb[:, :], e_d.ap())
        sv_sb = consts.tile([128, 1], f32)
        nc.sync.dma_start(sv_sb[:, :], sv_d.ap())
        bv_sb = consts.tile([128, 1], f32)
        nc.sync.dma_start(bv_sb[:, :], bv_d.ap())
        b2_sb = consts.tile([128, 1], f32)
        nc.sync.dma_start(b2_sb[:, :], b2_d.ap())

        # im2col tiles per z-plane pair q: R_q[(ci,t,dy), contiguous 9600] where
        # row (ci,t,dy) = xpad[ci, 2q+t, dy:dy+96, :] flattened (y-stride 100).
        rtiles = {}

        def load_rtile(q):
            rt = rpool.tile([KROW, S * XW], f16, name=f"rt{q}", tag="rt")
            nc.sync.dma_start(rt[:, :], xim_d.ap()[q])
            rtiles[q] = rt

        for zb2 in range(NZP):          # z-pair index; zb = 2*zb2
            for j in range(3):
                q = zb2 + j
                if q not in rtiles:
                    load_rtile(q)
            rqs = [rtiles[zb2 + j] for j in range(3)]
            rviews = [
                rq[:, :].rearrange("p (y x) -> p y x", y=S, x=XW) for rq in rqs
            ]
            zoff = 2 * zb2 * XM_Z
            for g in range(NGRP):
                # per-half-z-pair x input / output staging (one DMA per half)
                HHALF = S * S // 2
                h, gh = divmod(g, NGRP // 2)
                if gh == 0:
                    hoff = zoff + h * HHALF
                    xz = xpool.tile([2 * C, HHALF], f32,
                                    name=f"xz{zb2}_{h}", tag="xz")
                    nc.sync.dma_start(
                        xz[:, :],
                        bass.AP(tensor=xmod_d, offset=hoff,
                                ap=[[XM_C, C], [XM_Z, 2], [1, HHALF]]),
                    )
                    oz = opool.tile([2 * C, HHALF], f32,
                                    name=f"oz{zb2}_{h}", tag="oz")
                ptiles = [pconv.tile([128, FREE], f32, name=f"pc{i}", tag="pc")
                          for i in range(GRP)]
                for si in range(NSET):
                    j, dx = divmod(si, K)
                    lhsT = w_sb[:, si * 128:(si + 1) * 128]
                    for i in range(GRP):
                        yb = g * GRP + i
                        rhs = rviews[j][0:KROW, yb * YB:(yb + 1) * YB, dx:dx + S]
                        nc.tensor.matmul(
                            ptiles[i][:, :], lhsT, rhs,
                            start=(si == 0), stop=(si == NSET - 1),
                        )
                for i in range(GRP):
                    yb = g * GRP + i
                    u2 = u2pool.tile([128, FREE], f32)
                    nc.scalar.activation(
                        u2[:, :], ptiles[i][:, :], AF.Square,
                        bias=bv_sb[:, 0:1], scale=sv_sb[:, 0:1],
                    )
                    gt = gpool.tile([128, FREE], f16)
                    nc.scalar.activation(
                        gt[:, :], u2[:, :], AF.Exp,
                        bias=b2_sb[:, 0:1], scale=-1.0,
                    )
                    fs = pseg.tile([2 * C, FREE], f32)
                    nc.tensor.matmul(
                        fs[:, :], e_sb[:, :], gt[:, :], start=True, stop=True,
                    )
                    lyb = yb - h * (NYB // 2)
                    ysl = slice(lyb * YB * S, (lyb + 1) * YB * S)
                    nc.vector.tensor_add(oz[:, ysl], fs[:, :], xz[:, ysl])
                    nc.vector.tensor_scalar(
                        oz[:, ysl], oz[:, ysl], 0.0, MAXP,
                        op0=ALU.max, op1=ALU.min,
                    )
                if gh == NGRP // 2 - 1:
                    nc.sync.dma_start(
                        bass.AP(tensor=out_d, offset=hoff,
                                ap=[[XM_C, C], [XM_Z, 2], [1, HHALF]]),
                        oz[:, :],
                    )
    nc.compile()
    return nc


def _host_prep(x, kernels, m, s, T, c0, c1):
    x = np.asarray(x, np.float32)
    kernels = np.asarray(kernels, np.float32)
    m = np.asarray(m, np.float32)
    s = np.asarray(s, np.float32)
    Tf = np.float32(T)
    c0 = np.asarray(c0).astype(np.int64)
    c1 = np.asarray(c1).astype(np.int64)

    xt = np.ascontiguousarray(np.moveaxis(x[0], -1, 0))            # [9,96,96,96]
    ksum = kernels.sum(axis=(1, 2, 3), keepdims=True)
    kn = np.where(ksum > 0, kernels / ksum, kernels).astype(np.float32)

    xpad = np.pad(xt, ((0, 0), (PAD, PAD), (PAD, PAD), (PAD, PAD)),
                  mode="wrap").astype(np.float16)                  # [9,100,100,100]

    cnt = np.zeros(C, np.float32)
    for p in range(P):
        cnt[c1[p]] += 1.0
    xmod = (xt - (cnt / Tf)[:, None, None, None]).astype(np.float32)

    # Weight matrix: W[(ci,t,dy), (j,dx), (2p+zo)] = kn[p, 2j+t-zo, dy, dx]
    # when c0[p]==ci and 0 <= 2j+t-zo <= 4, else 0.
    W = np.zeros((KROW, NSET, 128), np.float32)
    for p in range(P):
        ci = int(c0[p])
        for j in range(3):
            for t in range(2):
                for zo in range(2):
                    d = 2 * j + t - zo
                    if 0 <= d <= 4:
                        W[ci * 10 + t * 5:ci * 10 + t * 5 + 5, j * 5:j * 5 + 5,
                          2 * p + zo] = kn[p, d]
    W = W.astype(np.float16)

    E = np.zeros((128, 2 * C), np.float16)
    for p in range(P):
        for zo in range(2):
            E[2 * p + zo, 2 * int(c1[p]) + zo] = 1.0

    a = 1.0 / (s * np.sqrt(np.float32(2.0)))
    sv = np.zeros((128, 1), np.float32)
    bv = np.zeros((128, 1), np.float32)
    sv[0::2, 0] = a
    sv[1::2, 0] = a
    bv[0::2, 0] = -m * a
    bv[1::2, 0] = -m * a
    b2 = np.full((128, 1), np.log(np.float32(2.0) / Tf), np.float32)

    in_maps = []
    for k in range(NCORES):
        slab = xpad[:, ZS * k:ZS * k + SLAB]                       # [9,16,100,100]
        # im2col: xim[q, ci*10 + t*5 + dy, :] = slab[ci, 2q+t].flat[dy*100:][:9600]
        planes = slab.reshape(C, SLAB // 2, 2, XW * XW)            # [9,8,2,10000]
        xim = np.empty((SLAB // 2, KROW, S * XW), np.float16)
        for dy in range(K):
            r = planes[:, :, :, dy * XW:dy * XW + S * XW]          # [9,8,2,9600]
            xim[:, :, :].reshape(SLAB // 2, C, 2, K, S * XW)[:, :, :, dy] = (
                r.transpose(1, 0, 2, 3))
        in_maps.append({
            "xim": xim,
            "xmod": np.ascontiguousarray(xmod[:, ZS * k:ZS * k + ZS]),
            "wmat": W,
            "emat": E,
            "svec": sv,
            "bvec": bv,
            "b2vec": b2,
        })
    return in_maps


_NC_CACHE = {}


def _get_nc():
    if "nc" not in _NC_CACHE:
        _NC_CACHE["nc"] = _build_nc()
    return _NC_CACHE["nc"]


def _gather(results):
    out = np.concatenate([results[k]["out"] for k in range(NCORES)], axis=1)
    return np.ascontiguousarray(np.moveaxis(out, 0, -1))[None]     # [1,96,96,96,9]


def kernel(x, kernels, m, s, T, c0, c1):
    from concourse import bass_utils

    nc = _get_nc()
    in_maps = _host_prep(x, kernels, m, s, T, c0, c1)
    res = bass_utils.run_bass_kernel_spmd(nc, in_maps, list(range(NCORES)))
    return _gather(res.results)
